# revision 1
# baseline (speedup 1.0000x reference)
"""GateRow kernel for Trainium2 (8 NeuronCores, SPMD data-parallel over batch).

Problem: out[b, g] = gates[g, 2*x[b, c0[g]] + x[b, c1[g]]]
  x: [16384, 8192] bool, gates: [8192, 4] bool, choices: [8192, 2] int32.

Strategy (per core, batch-sharded BS=2048):
  host:  build a doubled lookup table TAB = [x^T ; ~x^T ; ones ; zeros]
         (uint8, one row per input wire, BS bytes per row).  Classify each
         gate's 4-entry truth table into  out = (s>=t1) ^ (s>=t2)  with
         s = va + vb, where va/vb are the (possibly inverted / constant)
         gathered operand rows.  This covers all 16 boolean functions.
  device:
    1. dma_gather rows of TAB -> operand tiles [128 gates, BS] uint8
    2. one fused custom-DVE pass: l = (a+b >= t1) ^ (a+b >= t2) -> bf16
    3. PE transpose (identity matmul) [128,128] tiles -> PSUM f32
    4. ACT copies PSUM -> SBUF uint8 (cast)
    5. DMA out rows [b, g] (contiguous per batch row)
"""

import sys

for _p in ("/opt/trn_rl_repo", "/opt/pypackages"):
    if _p not in sys.path:
        sys.path.append(_p)

from contextlib import ExitStack

import numpy as np
import ml_dtypes

import concourse.bass as bass
import concourse.bacc as bacc
import concourse.tile as tile
import concourse.mybir as mybir
from concourse.bass_utils import run_bass_kernel_spmd

B, N, G, NCORES = 16384, 8192, 8192, 8
BS = B // NCORES  # 2048 batch rows per core

# ---------------------------------------------------------------------------
# Gate classification: truth table (4 bits, bit (2a+b)) ->
#   (fa, fb, t1, t2) with fa/fb in {0: v, 1: ~v, 2: one, 3: zero}
#   such that f(a,b) == ((va+vb) >= t1) ^ ((va+vb) >= t2)
# ---------------------------------------------------------------------------


def _classify_gates():
    forms = np.zeros((16, 4), dtype=np.int64)
    for tt in range(16):
        found = False
        for fa in range(4):
            for fb in range(4):
                for t1 in range(4):
                    for t2 in range(4):
                        ok = True
                        for a in (0, 1):
                            for b in (0, 1):
                                va = (a, 1 - a, 1, 0)[fa]
                                vb = (b, 1 - b, 1, 0)[fb]
                                s = va + vb
                                v = int(s >= t1) ^ int(s >= t2)
                                if v != ((tt >> (2 * a + b)) & 1):
                                    ok = False
                        if ok and not found:
                            forms[tt] = (fa, fb, t1, t2)
                            found = True
        assert found, f"truth table {tt} not representable"
    return forms


_FORMS = _classify_gates()

# ---------------------------------------------------------------------------
# Custom DVE op:  out = ((in0+in1) >= s0) ^ ((in0+in1) >= s1)
# ---------------------------------------------------------------------------

_GATE_LUT_OP = None


def _register_gate_lut():
    global _GATE_LUT_OP
    if _GATE_LUT_OP is not None:
        return _GATE_LUT_OP
    import concourse.dve_ops as dve_ops_mod
    from concourse.dve_ops import DveOp
    from concourse.dve_spec import Spec, Src0, Src1, C0, C1, lower, _has_src1
    from concourse.dve_uop import DveOpSpec

    name = "GATE_LUT_ANT"
    if any(op.name == name for op in dve_ops_mod.OPS):
        _GATE_LUT_OP = next(op for op in dve_ops_mod.OPS if op.name == name)
        return _GATE_LUT_OP

    s = Src0 + Src1
    spec = Spec(
        body=(s >= C0) ^ (s >= C1),
        reference=lambda in0, in1, s0, s1, imm2: (
            ((in0 + in1) >= s0) != ((in0 + in1) >= s1)
        ).astype(np.float32),
    )
    row = dve_ops_mod._CUSTOM_DVE_ROW_BASE + len(dve_ops_mod.OPS)
    dve_ops_mod._SUB_OPCODE_FOR_NAME[name] = row
    shas = {}
    for ver in ("v3", "v4"):
        uops = lower(spec, ver=ver)
        shas[ver] = DveOpSpec(
            name=name, opcode=row, uops=uops, rd1_en=_has_src1(spec)
        ).sha(ver)
    op = DveOp(name, spec, subdim=False, uops_sha=shas)
    dve_ops_mod.OPS.append(op)
    dve_ops_mod.CUSTOM_DVE_SPECS[name] = spec
    _GATE_LUT_OP = op
    return op


# ---------------------------------------------------------------------------
# Device program builder (parameterized so a small version can be simulated)
# ---------------------------------------------------------------------------


def build_nc(bs=BS, n=N, g=G, group=8, ncores=NCORES):
    """One SPMD program; all cores run it on their own batch shard."""
    lut_op = _register_gate_lut()
    nblk = g // 128          # gate blocks of 128
    ngrp = nblk // group     # gather groups
    ntab = 2 * n + 2         # x^T rows, ~x^T rows, ones row, zeros row
    mtiles = bs // 128       # batch sub-tiles per core
    nidx = group * 128       # indices per dma_gather call
    percall = nidx // 16     # int16s per partition per call

    nc = bacc.Bacc(
        "TRN2", target_bir_lowering=False, debug=False, num_devices=ncores
    )
    tab = nc.dram_tensor("tab", [ntab, bs], mybir.dt.uint8, kind="ExternalInput")
    idxs = nc.dram_tensor(
        "idxs", [128, 2 * ngrp * percall], mybir.dt.int16, kind="ExternalInput"
    )
    cst = nc.dram_tensor("cst", [128, 2 * nblk], mybir.dt.float32, kind="ExternalInput")
    ident = nc.dram_tensor("ident", [128, 128], mybir.dt.bfloat16, kind="ExternalInput")
    outd = nc.dram_tensor("out", [bs, g], mybir.dt.uint8, kind="ExternalOutput")

    with tile.TileContext(nc) as tc, ExitStack() as ctx:
        pconst = ctx.enter_context(tc.tile_pool(name="const", bufs=1))
        pgather = ctx.enter_context(tc.tile_pool(name="gather", bufs=2))
        pl = ctx.enter_context(tc.tile_pool(name="lut", bufs=2))
        posb = ctx.enter_context(tc.tile_pool(name="osb", bufs=2))
        pps = ctx.enter_context(tc.tile_pool(name="ps", bufs=4, space="PSUM"))

        idx_t = pconst.tile([128, idxs.shape[1]], mybir.dt.int16)
        nc.sync.dma_start(idx_t[:], idxs[:])
        cst_t = pconst.tile([128, 2 * nblk], mybir.dt.float32)
        nc.sync.dma_start(cst_t[:], cst[:])
        id_t = pconst.tile([128, 128], mybir.dt.bfloat16)
        nc.sync.dma_start(id_t[:], ident[:])

        for gi in range(ngrp):
            a_t = pgather.tile([128, group, bs], mybir.dt.uint8, tag="a")
            b_t = pgather.tile([128, group, bs], mybir.dt.uint8, tag="b")
            off = gi * 2 * percall
            nc.gpsimd.dma_gather(
                a_t[:],
                tab[:],
                idx_t[:, off : off + percall],
                nidx,
                nidx,
                bs,
                single_packet=False,
            )
            nc.gpsimd.dma_gather(
                b_t[:],
                tab[:],
                idx_t[:, off + percall : off + 2 * percall],
                nidx,
                nidx,
                bs,
                single_packet=False,
            )
            ls = []
            for j in range(group):
                bk = gi * group + j
                l_t = pl.tile([128, bs], mybir.dt.bfloat16, tag=f"l{j}")
                nc.vector._custom_dve(
                    lut_op,
                    out=l_t[:],
                    in0=a_t[:, j, :],
                    in1=b_t[:, j, :],
                    s0=cst_t[:, bk : bk + 1],
                    s1=cst_t[:, nblk + bk : nblk + bk + 1],
                )
                ls.append(l_t)
            for m in range(mtiles):
                osb = posb.tile([128, group * 128], mybir.dt.uint8, tag=f"o{m}")
                ps = pps.tile([128, group * 128], mybir.dt.bfloat16)
                for j in range(group):
                    nc.tensor.transpose(
                        ps[:, j * 128 : (j + 1) * 128],
                        ls[j][:, m * 128 : (m + 1) * 128],
                        id_t[:],
                    )
                nc.scalar.activation(
                    osb[:], ps[:], mybir.ActivationFunctionType.Copy
                )
                nc.sync.dma_start(
                    outd[
                        m * 128 : (m + 1) * 128,
                        gi * group * 128 : (gi + 1) * group * 128,
                    ],
                    osb[:],
                )
    nc.compile()
    return nc


# ---------------------------------------------------------------------------
# Host-side input prep
# ---------------------------------------------------------------------------


def _prep_inputs(x, gates, choices, bs=BS, n=N, g=G, group=8, ncores=NCORES):
    nblk = g // 128
    ngrp = nblk // group
    x8 = np.asarray(x, dtype=np.uint8)
    gates8 = np.asarray(gates, dtype=np.uint8)
    ch = np.asarray(choices, dtype=np.int64)

    tt = (gates8 << np.arange(4, dtype=np.uint8)).sum(axis=1).astype(np.int64)
    fa, fb, t1, t2 = (_FORMS[tt, k] for k in range(4))

    # operand row index in TAB for each gate
    ia = np.where(fa <= 1, ch[:, 0] + fa * n, 2 * n + (fa - 2))
    ib = np.where(fb <= 1, ch[:, 1] + fb * n, 2 * n + (fb - 2))
    assert ia.max() < 2 * n + 2 and ib.max() < 2 * n + 2

    # dma_gather wrapped index layout: per call, idx i -> partition i%16,
    # slot i//16; replicated across the 8 gpsimd cores (x8 partitions).
    cols = []
    for gi in range(ngrp):
        for arr in (ia, ib):
            flat = arr[gi * group * 128 : (gi + 1) * group * 128].astype(np.int16)
            wrapped = flat.reshape(-1, 16).T  # [16, nidx/16]
            cols.append(np.tile(wrapped, (8, 1)))  # [128, nidx/16]
    idxs_np = np.ascontiguousarray(np.concatenate(cols, axis=1))

    # thresholds, [128, 2*nblk] f32; column bk = t1 of gates bk*128..bk*128+127
    t1m = t1.reshape(nblk, 128).T.astype(np.float32)
    t2m = t2.reshape(nblk, 128).T.astype(np.float32)
    cst_np = np.ascontiguousarray(np.concatenate([t1m, t2m], axis=1))

    ident_np = np.eye(128, dtype=ml_dtypes.bfloat16)

    # doubled table
    xt = x8.T  # [n, B] view
    ntab = 2 * n + 2
    in_maps = []
    for k in range(ncores):
        sl = slice(k * bs, (k + 1) * bs)
        tabk = np.empty((ntab, bs), dtype=np.uint8)
        tabk[:n] = xt[:, sl]
        tabk[n : 2 * n] = 1 - tabk[:n]
        tabk[2 * n] = 1
        tabk[2 * n + 1] = 0
        in_maps.append(
            {"tab": tabk, "idxs": idxs_np, "cst": cst_np, "ident": ident_np}
        )
    return in_maps


# ---------------------------------------------------------------------------
# Entry point
# ---------------------------------------------------------------------------

_NC_CACHE = {}


def _get_nc(key=(BS, N, G, 8)):
    if key not in _NC_CACHE:
        _NC_CACHE[key] = build_nc(*key)
    return _NC_CACHE[key]


def kernel(x, gates, choices):
    in_maps = _prep_inputs(x, gates, choices)
    nc = _get_nc()
    res = run_bass_kernel_spmd(nc, in_maps, list(range(NCORES)))
    out = np.concatenate([res.results[k]["out"] for k in range(NCORES)], axis=0)
    return out.astype(bool)



# revision 3
# speedup vs baseline: 5.4499x; 5.4499x over previous
"""GateRow kernel for Trainium2 (8 NeuronCores, SPMD, gate-sharded, bit-packed).

Problem: out[b, g] = gates[g, 2*x[b, c0[g]] + x[b, c1[g]]]
  x: [16384, 8192] bool, gates: [8192, 4] bool, choices: [8192, 2] int32.

Strategy:
  Every 2-input boolean gate is  rowA OP rowB  for OP in {AND, OR, XOR}
  once operand inversion and constants are absorbed into a doubled
  lookup table TAB = [x^T ; ~x^T ; ones ; zeros] (one row per wire).
  Bit-pack the batch dimension (8 rows/byte) so each TAB row is
  B/8 = 2048 bytes and the boolean op is a plain bitwise uint8 op.

  Shard by GATES: core k owns 1024 gates.  Host sorts gates into
  type-homogeneous blocks of 128 under a fixed per-core schedule
  (3 AND blocks, 3 OR blocks, 2 XOR blocks); "flexible" gates
  (constants / projections, expressible in any family) pad the
  buckets to exact capacity.  The host un-permutes output columns.

  Device (per core): 4 dma_gathers (512 rows each, 2048 B/row, 4 MB
  total), 8 stock tensor_tensor bitwise ops on the DVE, 8 output DMAs
  (2 MB total).  No PE, no PSUM, no custom ops.
"""

import sys

for _p in ("/opt/trn_rl_repo", "/opt/pypackages"):
    if _p not in sys.path:
        sys.path.append(_p)

from contextlib import ExitStack

import numpy as np

import concourse.bass as bass
import concourse.bacc as bacc
import concourse.tile as tile
import concourse.mybir as mybir
from concourse.bass_utils import run_bass_kernel_spmd

B, N, G, NCORES = 16384, 8192, 8192, 8
GPC = G // NCORES           # 1024 gates per core
NBLK = GPC // 128           # 8 gate blocks per core
PB = B // 8                 # 2048 packed bytes per table row
ROW_ONE = 2 * N             # all-ones table row
ROW_ZERO = 2 * N + 1        # all-zeros table row

# Per-core block op schedule: 3 AND, 3 OR, 2 XOR blocks of 128 gates.
SCHED = ("and",) * 3 + ("or",) * 3 + ("xor",) * 2
CAP = {"and": 3 * 128 * NCORES, "or": 3 * 128 * NCORES, "xor": 2 * 128 * NCORES}

# ---------------------------------------------------------------------------
# Gate classification.
#   tt bit (2a+b) = f(a, b).  Operand selectors:
#     0: x[c0]   1: ~x[c0]   2: x[c1]   3: ~x[c1]   4: ones   5: zeros
#   SEL[op][tt] = (selA, selB) with f == rowA op rowB; None if inexpressible.
# ---------------------------------------------------------------------------

_OPS = ("and", "or", "xor")
_NPOP = {"and": np.bitwise_and, "or": np.bitwise_or, "xor": np.bitwise_xor}


def _build_sel():
    sel = {op: [None] * 16 for op in _OPS}
    for tt in range(16):
        for op in _OPS:
            for sa in range(6):
                for sb in range(6):
                    ok = True
                    for a in (0, 1):
                        for b in (0, 1):
                            va = (a, 1 - a, b, 1 - b, 1, 0)[sa]
                            vb = (a, 1 - a, b, 1 - b, 1, 0)[sb]
                            r = int(_NPOP[op](va, vb))
                            if r != ((tt >> (2 * a + b)) & 1):
                                ok = False
                    if ok and sel[op][tt] is None:
                        sel[op][tt] = (sa, sb)
    return sel


_SEL = _build_sel()
# Which families can express each tt (for bucket assignment).
_FAMS = [frozenset(op for op in _OPS if _SEL[op][tt] is not None) for tt in range(16)]


# ---------------------------------------------------------------------------
# Device program
# ---------------------------------------------------------------------------

_ALU = {
    "and": mybir.AluOpType.bitwise_and,
    "or": mybir.AluOpType.bitwise_or,
    "xor": mybir.AluOpType.bitwise_xor,
}


def build_nc(ncores=NCORES):
    """One SPMD program; all cores run it on their own gate shard."""
    nhalf = NBLK // 2        # blocks per pipeline half
    nidx = nhalf * 128       # rows per dma_gather call
    percall = nidx // 16     # int16s per partition per call

    nc = bacc.Bacc(
        "TRN2", target_bir_lowering=False, debug=False, num_devices=ncores
    )
    tab = nc.dram_tensor("tab", [2 * N + 2, PB], mybir.dt.uint8, kind="ExternalInput")
    idxs = nc.dram_tensor(
        "idxs", [128, 4 * percall], mybir.dt.int16, kind="ExternalInput"
    )
    outd = nc.dram_tensor("out", [GPC, PB], mybir.dt.uint8, kind="ExternalOutput")

    with tile.TileContext(nc) as tc, ExitStack() as ctx:
        pconst = ctx.enter_context(tc.tile_pool(name="const", bufs=1))
        pg = ctx.enter_context(tc.tile_pool(name="gather", bufs=2))
        po = ctx.enter_context(tc.tile_pool(name="osb", bufs=2))

        idx_t = pconst.tile([128, idxs.shape[1]], mybir.dt.int16)
        nc.sync.dma_start(idx_t[:], idxs[:])

        for h in range(2):
            a_t = pg.tile([128, nhalf, PB], mybir.dt.uint8, tag="a")
            b_t = pg.tile([128, nhalf, PB], mybir.dt.uint8, tag="b")
            off = h * 2 * percall
            nc.gpsimd.dma_gather(
                a_t[:],
                tab[:],
                idx_t[:, off : off + percall],
                nidx,
                nidx,
                PB,
                single_packet=False,
            )
            nc.gpsimd.dma_gather(
                b_t[:],
                tab[:],
                idx_t[:, off + percall : off + 2 * percall],
                nidx,
                nidx,
                PB,
                single_packet=False,
            )
            for j in range(nhalf):
                bk = h * nhalf + j
                o_t = po.tile([128, PB], mybir.dt.uint8, tag=f"o{bk}")
                nc.vector.tensor_tensor(
                    o_t[:], a_t[:, j, :], b_t[:, j, :], op=_ALU[SCHED[bk]]
                )
                nc.sync.dma_start(outd[bk * 128 : (bk + 1) * 128, :], o_t[:])
    nc.compile()
    return nc


# ---------------------------------------------------------------------------
# Host-side input prep
# ---------------------------------------------------------------------------


def _prep(x, gates, choices):
    x8 = np.asarray(x, dtype=np.uint8)
    gates8 = np.asarray(gates, dtype=np.uint8)
    ch = np.asarray(choices, dtype=np.int64)

    # Packed doubled table (replicated on every core).
    xp = np.packbits(x8, axis=0)              # [B/8, N], bit MSB = lowest batch row
    tab = np.empty((2 * N + 2, PB), dtype=np.uint8)
    tab[:N] = xp.T
    tab[N : 2 * N] = ~tab[:N]
    tab[ROW_ONE] = 0xFF
    tab[ROW_ZERO] = 0x00

    # Bucket assignment: required-family gates first, flexible gates pad.
    tt = (gates8 << np.arange(4, dtype=np.uint8)).sum(axis=1).astype(np.int64)
    req = {op: [t for t in range(16) if _FAMS[t] == {op}] for op in _OPS}
    flex = [t for t in range(16) if len(_FAMS[t]) == 3]
    assert sum(len(v) for v in req.values()) + len(flex) == 16

    gid = np.arange(G)
    flex_pool = gid[np.isin(tt, flex)]
    fp = 0
    slots = {}
    for op in _OPS:
        need = gid[np.isin(tt, req[op])]
        pad = CAP[op] - len(need)
        assert pad >= 0, f"bucket {op} overflow: {len(need)} > {CAP[op]}"
        slots[op] = np.concatenate([need, flex_pool[fp : fp + pad]])
        fp += pad
    assert fp == len(flex_pool)

    # Device gate order (core-major, schedule-major) + operand row indices.
    npc = {"and": 3 * 128, "or": 3 * 128, "xor": 2 * 128}
    perm = np.empty(G, dtype=np.int64)        # device row -> gate id
    ia = np.empty(G, dtype=np.int64)
    ib = np.empty(G, dtype=np.int64)
    r = 0
    for k in range(ncr := NCORES):
        for op in _OPS:
            g = slots[op][k * npc[op] : (k + 1) * npc[op]]
            lut = [_SEL[op][t] or (5, 5) for t in range(16)]  # (5,5) never used
            selA = np.array([s[0] for s in lut])[tt[g]]
            selB = np.array([s[1] for s in lut])[tt[g]]
            rows = np.stack(
                [ch[g, 0], ch[g, 0] + N, ch[g, 1], ch[g, 1] + N,
                 np.full(len(g), ROW_ONE), np.full(len(g), ROW_ZERO)]
            )
            n = len(g)
            perm[r : r + n] = g
            ia[r : r + n] = rows[selA, np.arange(n)]
            ib[r : r + n] = rows[selB, np.arange(n)]
            r += n
    assert r == G

    # Wrapped int16 index layout per core: 4 calls x (nhalf*128) rows;
    # call order [A half0, B half0, A half1, B half1].
    nhalf = NBLK // 2
    in_maps = []
    for k in range(NCORES):
        s = slice(k * GPC, (k + 1) * GPC)
        iak = ia[s].reshape(NBLK, 128)
        ibk = ib[s].reshape(NBLK, 128)
        cols = []
        for h in range(2):
            for arr in (iak, ibk):
                flat = arr[h * nhalf : (h + 1) * nhalf].reshape(-1).astype(np.int16)
                wrapped = flat.reshape(-1, 16).T      # [16, nidx/16]
                cols.append(np.tile(wrapped, (8, 1)))  # [128, nidx/16]
        idxs_np = np.ascontiguousarray(np.concatenate(cols, axis=1))
        in_maps.append({"tab": tab, "idxs": idxs_np})
    return in_maps, perm


# ---------------------------------------------------------------------------
# Entry point
# ---------------------------------------------------------------------------

_NC_CACHE = {}


def _get_nc():
    if "nc" not in _NC_CACHE:
        _NC_CACHE["nc"] = build_nc()
    return _NC_CACHE["nc"]


def kernel(x, gates, choices):
    in_maps, perm = _prep(x, gates, choices)
    nc = _get_nc()
    res = run_bass_kernel_spmd(nc, in_maps, list(range(NCORES)))
    packed = np.concatenate([res.results[k]["out"] for k in range(NCORES)], axis=0)
    ordered = np.empty_like(packed)
    ordered[perm] = packed                    # un-permute gate rows
    up = np.unpackbits(ordered, axis=1)       # [G, B] 0/1 uint8
    return up.view(np.bool_).T                # [B, G] bool view


# revision 4
# speedup vs baseline: 6.3183x; 1.1593x over previous
"""GateRow kernel for Trainium2 (8 NeuronCores, SPMD, gate-sharded, bit-packed).

Problem: out[b, g] = gates[g, 2*x[b, c0[g]] + x[b, c1[g]]]
  x: [16384, 8192] bool, gates: [8192, 4] bool, choices: [8192, 2] int32.

Strategy:
  Every 2-input boolean gate is  rowA OP rowB  for OP in {AND, OR, XOR}
  once operand inversion and constants are absorbed into a doubled
  lookup table TAB = [x^T ; ~x^T ; ones ; zeros] (one row per wire).
  Bit-pack the batch dimension (8 rows/byte) so each TAB row is
  B/8 = 2048 bytes and the boolean op is a plain bitwise op (done on
  uint32 views: bitwise is byte-local, and 32-bit elements quarter the
  DVE element count).

  Shard by GATES: core k owns 1024 gates.  Host sorts gates into
  type-homogeneous blocks of 128 under a fixed per-core schedule
  (3 AND blocks, 3 OR blocks, 2 XOR blocks); "flexible" gates
  (constants / projections, expressible in any family) pad the
  buckets to exact capacity.  The host un-permutes output columns.

  Device (per core): dma_gathers (2048 rows, 2048 B/row, 4 MB total),
  8 stock tensor_tensor bitwise ops, 8 output DMAs (2 MB total).
  No PE, no PSUM, no custom DVE ops.
"""

import sys

for _p in ("/opt/trn_rl_repo", "/opt/pypackages"):
    if _p not in sys.path:
        sys.path.append(_p)

from contextlib import ExitStack

import numpy as np

import concourse.bass as bass
import concourse.bacc as bacc
import concourse.tile as tile
import concourse.mybir as mybir
from concourse.bass_utils import run_bass_kernel_spmd

B, N, G, NCORES = 16384, 8192, 8192, 8
GPC = G // NCORES           # 1024 gates per core
NBLK = GPC // 128           # 8 gate blocks per core
PB = B // 8                 # 2048 packed bytes per table row
PW = PB // 4                # 512 packed uint32 words per table row
ROW_ONE = 2 * N             # all-ones table row
ROW_ZERO = 2 * N + 1        # all-zeros table row
NCALLS = 4                  # dma_gather calls (a+b interleaved per call)

# Per-core block op schedule: 3 AND, 3 OR, 2 XOR blocks of 128 gates.
SCHED = ("and",) * 3 + ("or",) * 3 + ("xor",) * 2
CAP = {"and": 3 * 128 * NCORES, "or": 3 * 128 * NCORES, "xor": 2 * 128 * NCORES}

# ---------------------------------------------------------------------------
# Gate classification.
#   tt bit (2a+b) = f(a, b).  Operand selectors:
#     0: x[c0]   1: ~x[c0]   2: x[c1]   3: ~x[c1]   4: ones   5: zeros
#   SEL[op][tt] = (selA, selB) with f == rowA op rowB; None if inexpressible.
# ---------------------------------------------------------------------------

_OPS = ("and", "or", "xor")
_NPOP = {"and": np.bitwise_and, "or": np.bitwise_or, "xor": np.bitwise_xor}


def _build_sel():
    sel = {op: [None] * 16 for op in _OPS}
    for tt in range(16):
        for op in _OPS:
            for sa in range(6):
                for sb in range(6):
                    ok = True
                    for a in (0, 1):
                        for b in (0, 1):
                            va = (a, 1 - a, b, 1 - b, 1, 0)[sa]
                            vb = (a, 1 - a, b, 1 - b, 1, 0)[sb]
                            r = int(_NPOP[op](va, vb))
                            if r != ((tt >> (2 * a + b)) & 1):
                                ok = False
                    if ok and sel[op][tt] is None:
                        sel[op][tt] = (sa, sb)
    return sel


_SEL = _build_sel()
_FAMS = [frozenset(op for op in _OPS if _SEL[op][tt] is not None) for tt in range(16)]


# ---------------------------------------------------------------------------
# Device program
# ---------------------------------------------------------------------------

_ALU = {
    "and": mybir.AluOpType.bitwise_and,
    "or": mybir.AluOpType.bitwise_or,
    "xor": mybir.AluOpType.bitwise_xor,
}


def build_nc(ncalls=NCALLS, ncores=NCORES):
    """One SPMD program; all cores run it on their own gate shard.

    ncalls dma_gather calls; each gathers the A then B rows for
    NBLK/ncalls consecutive gate blocks (interleaved a,b per call group
    so compute on group i overlaps the gather of group i+1).
    """
    npc = NBLK // ncalls     # gate blocks per call group
    nidx = npc * 2 * 128     # rows per dma_gather call (a rows then b rows)
    percall = nidx // 16     # int16s per partition per call

    nc = bacc.Bacc(
        "TRN2", target_bir_lowering=False, debug=False, num_devices=ncores
    )
    tab = nc.dram_tensor("tab", [2 * N + 2, PW], mybir.dt.uint32, kind="ExternalInput")
    idxs = nc.dram_tensor(
        "idxs", [128, ncalls * percall], mybir.dt.int16, kind="ExternalInput"
    )
    outd = nc.dram_tensor("out", [GPC, PW], mybir.dt.uint32, kind="ExternalOutput")

    with tile.TileContext(nc) as tc, ExitStack() as ctx:
        pconst = ctx.enter_context(tc.tile_pool(name="const", bufs=1))
        pg = ctx.enter_context(tc.tile_pool(name="gather", bufs=2))
        po = ctx.enter_context(tc.tile_pool(name="osb", bufs=2))

        idx_t = pconst.tile([128, idxs.shape[1]], mybir.dt.int16)
        nc.sync.dma_start(idx_t[:], idxs[:])

        for h in range(ncalls):
            g_t = pg.tile([128, 2 * npc, PW], mybir.dt.uint32, tag="g")
            nc.gpsimd.dma_gather(
                g_t[:],
                tab[:],
                idx_t[:, h * percall : (h + 1) * percall],
                nidx,
                nidx,
                PW,
                single_packet=False,
            )
            for j in range(npc):
                bk = h * npc + j
                o_t = po.tile([128, PW], mybir.dt.uint32, tag=f"o{bk}")
                nc.vector.tensor_tensor(
                    o_t[:],
                    g_t[:, 2 * j, :],
                    g_t[:, 2 * j + 1, :],
                    op=_ALU[SCHED[bk]],
                )
                nc.sync.dma_start(outd[bk * 128 : (bk + 1) * 128, :], o_t[:])
    nc.compile()
    return nc


# ---------------------------------------------------------------------------
# Host-side input prep
# ---------------------------------------------------------------------------


def _prep(x, gates, choices, ncalls=NCALLS):
    x8 = np.asarray(x, dtype=np.uint8)
    gates8 = np.asarray(gates, dtype=np.uint8)
    ch = np.asarray(choices, dtype=np.int64)

    # Packed doubled table (replicated on every core).
    xp = np.packbits(x8, axis=0)              # [B/8, N], bit MSB = lowest batch row
    tab = np.empty((2 * N + 2, PB), dtype=np.uint8)
    tab[:N] = xp.T
    tab[N : 2 * N] = ~tab[:N]
    tab[ROW_ONE] = 0xFF
    tab[ROW_ZERO] = 0x00
    tab32 = tab.view(np.uint32)

    # Bucket assignment: required-family gates first, flexible gates pad.
    tt = (gates8 << np.arange(4, dtype=np.uint8)).sum(axis=1).astype(np.int64)
    req = {op: [t for t in range(16) if _FAMS[t] == {op}] for op in _OPS}
    flex = [t for t in range(16) if len(_FAMS[t]) == 3]
    assert sum(len(v) for v in req.values()) + len(flex) == 16

    gid = np.arange(G)
    flex_pool = gid[np.isin(tt, flex)]
    fp = 0
    slots = {}
    for op in _OPS:
        need = gid[np.isin(tt, req[op])]
        pad = CAP[op] - len(need)
        assert pad >= 0, f"bucket {op} overflow: {len(need)} > {CAP[op]}"
        slots[op] = np.concatenate([need, flex_pool[fp : fp + pad]])
        fp += pad
    assert fp == len(flex_pool)

    # Device gate order (core-major, schedule-major) + operand row indices.
    npcg = {"and": 3 * 128, "or": 3 * 128, "xor": 2 * 128}
    perm = np.empty(G, dtype=np.int64)        # device row -> gate id
    ia = np.empty(G, dtype=np.int64)
    ib = np.empty(G, dtype=np.int64)
    r = 0
    for k in range(NCORES):
        for op in _OPS:
            g = slots[op][k * npcg[op] : (k + 1) * npcg[op]]
            lut = [_SEL[op][t] or (5, 5) for t in range(16)]  # (5,5) never used
            selA = np.array([s[0] for s in lut])[tt[g]]
            selB = np.array([s[1] for s in lut])[tt[g]]
            rows = np.stack(
                [ch[g, 0], ch[g, 0] + N, ch[g, 1], ch[g, 1] + N,
                 np.full(len(g), ROW_ONE), np.full(len(g), ROW_ZERO)]
            )
            n = len(g)
            perm[r : r + n] = g
            ia[r : r + n] = rows[selA, np.arange(n)]
            ib[r : r + n] = rows[selB, np.arange(n)]
            r += n
    assert r == G

    # Wrapped int16 index layout per core: ncalls calls; call h covers
    # npc gate blocks -> flat order [a rows of npc blocks, b rows of npc
    # blocks] interleaved as [a(blk0),b(blk0),a(blk1),b(blk1),...] to
    # match tile slots (2j, 2j+1).
    npc = NBLK // ncalls
    in_maps = []
    for k in range(NCORES):
        s = slice(k * GPC, (k + 1) * GPC)
        iak = ia[s].reshape(NBLK, 128)
        ibk = ib[s].reshape(NBLK, 128)
        cols = []
        for h in range(ncalls):
            inter = np.empty((2 * npc, 128), dtype=np.int16)
            inter[0::2] = iak[h * npc : (h + 1) * npc]
            inter[1::2] = ibk[h * npc : (h + 1) * npc]
            flat = inter.reshape(-1)
            wrapped = flat.reshape(-1, 16).T      # [16, nidx/16]
            cols.append(np.tile(wrapped, (8, 1)))  # [128, nidx/16]
        idxs_np = np.ascontiguousarray(np.concatenate(cols, axis=1))
        in_maps.append({"tab": tab32, "idxs": idxs_np})
    return in_maps, perm


# ---------------------------------------------------------------------------
# Entry point
# ---------------------------------------------------------------------------

_NC_CACHE = {}


def _get_nc():
    if "nc" not in _NC_CACHE:
        _NC_CACHE["nc"] = build_nc()
    return _NC_CACHE["nc"]


def kernel(x, gates, choices):
    in_maps, perm = _prep(x, gates, choices)
    nc = _get_nc()
    res = run_bass_kernel_spmd(nc, in_maps, list(range(NCORES)))
    packed = np.concatenate(
        [res.results[k]["out"].view(np.uint8) for k in range(NCORES)], axis=0
    )
    ordered = np.empty_like(packed)
    ordered[perm] = packed                    # un-permute gate rows
    up = np.unpackbits(ordered, axis=1)       # [G, B] 0/1 uint8
    return up.view(np.bool_).T                # [B, G] bool view
